# revision 1
# baseline (speedup 1.0000x reference)
"""Trainium2 Bass kernel for nn_AttentionFlow (BiDAF-style attention flow).

Math (per batch b, all biases cancel):
  s[t,i]   = <c_t,w_c> + <q_i,w_q> + <c_t*q_i, w_cq>  (+ biases)
  a        = softmax_i(s)          -> c2q = a @ q
  beta     = softmax_t(max_i s)    -> q2c = beta^T c
  out      = [c | c2q | c*c2q | c*q2c]

Key identities:
  * softmax_i(s[t,:]) is invariant to the per-row term sc[t] and all biases.
  * beta depends only on sc_raw[t] + max_i(sq_raw[i] + scq_raw[t,i]).
  * sc is folded into the matmul rhs:  qa[d,i] = q^T[d,i]*w_cq[d] + w_c[d].
  * t and i orderings are arbitrary (softmax/sums are order-invariant and
    outputs are re-addressed by AP), so row->partition maps are chosen for
    DMA contiguity when opts['contig_in'] is set.

Sharding: data-parallel over batch, one batch element per NeuronCore (8).
"""

import numpy as np

N_CORES = 8
T, I, D = 2048, 512, 512
TT = T // 128  # 16 row tiles
KC = 4         # 128-chunks of D (and of I)

DEFAULT_OPTS = dict(contig_in=True, out_ring="sync", two_pass=False,
                    skip_out=False, memset_in=False, dma_pair=False,
                    bufs_work=3, bufs_out=3, out_split=False,
                    dup_pe=False, dup_dve=False, dup_act=False, bloat=0,
                    act_copies="act", exp_accum=False, g_accum=False,
                    mul_eng="gpsimd", batch_recip=False, alt_copies=False,
                    split_in=True, q2c_inline=False, o4_split=True,
                    ps_tr_bufs=2, ct_eng="dve", early_cout=False,
                    ps_s_bufs=2, ps_mm2_bufs=2, fine_tiles=True, fine_c=True)

_BUILT = None


def _build(reps=1, timing_mode=False, opts=None):
    import concourse.tile as tile
    from concourse import bacc, mybir
    from concourse.masks import make_identity

    o = dict(DEFAULT_OPTS)
    if opts:
        o.update(opts)

    f32 = mybir.dt.float32
    f32r = mybir.dt.float32r
    bf16 = mybir.dt.bfloat16
    AF = mybir.ActivationFunctionType
    AX = mybir.AxisListType
    ALU = mybir.AluOpType

    nc = bacc.Bacc("TRN2", target_bir_lowering=False, debug=False,
                   num_devices=N_CORES)
    c_d = nc.dram_tensor("c", [T, D], f32, kind="ExternalInput").ap()
    q_d = nc.dram_tensor("q", [I, D], f32, kind="ExternalInput").ap()
    wc_d = nc.dram_tensor("wc", [D], f32, kind="ExternalInput").ap()
    wq_d = nc.dram_tensor("wq", [D], f32, kind="ExternalInput").ap()
    wcq_d = nc.dram_tensor("wcq", [D], f32, kind="ExternalInput").ap()
    out_kind = "Internal" if timing_mode else "ExternalOutput"
    out_d = nc.dram_tensor("out", [T, 4 * D], f32, kind=out_kind).ap()
    tick_d = (nc.dram_tensor("tick", [1, 1], f32, kind="ExternalOutput").ap()
              if timing_mode else None)

    out_eng = {"sync": nc.sync, "scalar": nc.scalar}[o["out_ring"]]

    with tile.TileContext(nc) as tc:
        with (
            tc.tile_pool(name="const", bufs=1) as constp,
            tc.tile_pool(name="big", bufs=1) as bigp,
            tc.tile_pool(name="work", bufs=o["bufs_work"]) as workp,
            tc.tile_pool(name="outp", bufs=o["bufs_out"]) as outp,
            tc.tile_pool(name="ps_tr", bufs=o["ps_tr_bufs"],
                         space="PSUM") as ps_tr,
            tc.tile_pool(name="ps_acc", bufs=1, space="PSUM") as ps_acc,
            tc.tile_pool(name="ps_s", bufs=o["ps_s_bufs"],
                         space="PSUM") as ps_s,
            tc.tile_pool(name="ps_mm2", bufs=o["ps_mm2_bufs"],
                         space="PSUM") as ps_mm2,
        ):
            for _rep in range(reps):
                # ---------------- phase 0 -----------------------------------
                ident_f = constp.tile([128, 128], f32, tag="idf")
                make_identity(nc, ident_f[:])
                ident_b = constp.tile([128, 128], bf16, tag="idb")
                make_identity(nc, ident_b[:])
                ones_row_f = constp.tile([1, 128], f32, tag="ones_row_f")
                nc.vector.memset(ones_row_f[:], 1.0)
                ones_row = constp.tile([1, 128], f32r, tag="ones_row")
                nc.vector.tensor_copy(ones_row[:], ones_row_f[:])
                ones_col = constp.tile([128, 1], f32, tag="ones_col")
                nc.vector.memset(ones_col[:], 1.0)

                wcq_col = constp.tile([128, KC], f32, tag="wcq_col")
                nc.sync.dma_start(wcq_col[:],
                                  wcq_d.rearrange("(a b) -> b a", b=128))
                wc_col = constp.tile([128, KC], f32, tag="wc_col")
                nc.sync.dma_start(wc_col[:],
                                  wc_d.rearrange("(a b) -> b a", b=128))
                wq_col = constp.tile([128, KC], f32, tag="wq_col")
                nc.sync.dma_start(wq_col[:],
                                  wq_d.rearrange("(a b) -> b a", b=128))

                q_sb = bigp.tile([128, KC, D], f32, tag="q_sb")
                if o["memset_in"]:
                    nc.gpsimd.memset(q_sb[:], 0.01)
                elif o["contig_in"]:
                    nc.sync.dma_start(
                        q_sb[:], q_d.rearrange("(p k) d -> p k d", k=KC))
                else:
                    nc.sync.dma_start(
                        q_sb[:], q_d.rearrange("(k p) d -> p k d", p=128))
                q_bf = bigp.tile([128, KC, D], bf16, tag="q_bf")
                nc.vector.tensor_copy(q_bf[:], q_sb[:])

                c_sb = []
                if o["memset_in"]:
                    for jj in range(4):
                        t_ = bigp.tile([128, 4, D], f32, tag=f"c_sb{jj}")
                        nc.gpsimd.memset(t_[:], 0.02)
                        c_sb.append(t_)
                elif o["contig_in"] and o["fine_c"]:
                    crs = c_d.rearrange("(p j) d -> p j d", j=TT)
                    c_fine = []
                    for _j in range(TT):
                        cf = bigp.tile([128, D], f32, tag=f"cin{_j}")
                        nc.sync.dma_start(cf[:], crs[:, _j, :])
                        c_fine.append(cf)
                elif o["contig_in"]:
                    crs = c_d.rearrange("(p j) d -> p j d", j=TT)
                    if o["split_in"]:
                        for jj in range(4):
                            t_ = bigp.tile([128, 4, D], f32, tag=f"c_sb{jj}")
                            for jr in range(4):
                                nc.sync.dma_start(
                                    t_[:, jr:jr + 1, :],
                                    crs[:, 4 * jj + jr:4 * jj + jr + 1, :])
                            c_sb.append(t_)
                    else:
                        for jj in range(4):
                            t_ = bigp.tile([128, 4, D], f32, tag=f"c_sb{jj}")
                            nc.sync.dma_start(t_[:],
                                              crs[:, 4 * jj:4 * jj + 4, :])
                            c_sb.append(t_)
                else:
                    for jj in range(4):
                        t_ = bigp.tile([128, 4, D], f32, tag=f"c_sb{jj}")
                        nc.sync.dma_start(
                            t_[:],
                            c_d[jj * 512:(jj + 1) * 512, :].rearrange(
                                "(j p) d -> p j d", p=128))
                        c_sb.append(t_)

                if o["contig_in"]:
                    ors = out_d.rearrange("(p j) w -> p j w", j=TT)

                    def out_ap(j, sl):
                        return ors[:, j, sl]
                else:
                    def out_ap(j, sl):
                        return out_d[j * 128:(j + 1) * 128, sl]

                def c_tile(j):
                    if o["contig_in"] and o["fine_c"]:
                        return c_fine[j]
                    jj_, jr_ = divmod(j, 4)
                    return c_sb[jj_][:, jr_]

                _out_n = [0]

                def out_dma(j, sl, src):
                    if o["skip_out"]:
                        return
                    _out_n[0] += 1
                    eng = (nc.scalar if (o["out_split"] and _out_n[0] % 2)
                           else out_eng)
                    eng.dma_start(out_ap(j, sl), src)

                if o["dma_pair"]:
                    for j in range(TT):
                        jj, jr = divmod(j, 4)
                        cj = c_sb[jj][:, jr]
                        out_dma(j, slice(0, 512), cj[:])
                        out_dma(j, slice(512, 2048),
                                c_sb[jj][:].rearrange("p a d -> p (a d)")
                                [:, 0:1536])
                    continue

                def copy_op(dst, src):
                    if o["act_copies"] == "dve":
                        nc.vector.tensor_copy(dst, src)
                    else:
                        nc.scalar.copy(dst, src)

                if o["early_cout"] and not o["dma_pair"]:
                    for j in range(TT):
                        jj, jr = divmod(j, 4)
                        out_dma(j, slice(0, 512), c_sb[jj][:, jr])

                # Q^T, qa = Q^T * wcq + wc
                qt = bigp.tile([128, KC, I], f32, tag="qt")
                qa = bigp.tile([128, KC, I], f32r, tag="qa")
                for k in range(KC):
                    pt = ps_tr.tile([128, I], f32, tag="ps_tr")
                    for ik in range(KC):
                        nc.tensor.transpose(
                            pt[:, ik * 128:(ik + 1) * 128],
                            q_sb[:, ik, k * 128:(k + 1) * 128],
                            ident_f[:])
                    copy_op(qt[:, k], pt[:])
                    nc.vector.tensor_scalar(
                        qa[:, k], pt[:], wcq_col[:, k:k + 1],
                        wc_col[:, k:k + 1], op0=ALU.mult, op1=ALU.add)

                # sq_row[1, I] = w_q^T Q^T
                ps_sq = ps_s.tile([1, I], f32, tag="ps_s")
                for k in range(KC):
                    nc.tensor.matmul(ps_sq[:], wq_col[:, k:k + 1], qt[:, k],
                                     start=(k == 0), stop=(k == KC - 1))
                sq_row = constp.tile([1, I], f32r, tag="sq_row")
                copy_op(sq_row[:], ps_sq[:])

                scratch1 = constp.tile([1, 1], f32, tag="scratch1")
                g = constp.tile([128, TT], f32, tag="g")
                if o["q2c_inline"]:
                    psq2c = ps_acc.tile([1, D], f32, tag="ps_q2c")
                    psZ = ps_acc.tile([1, 1], f32, tag="ps_Z")
                mhat = constp.tile([128, TT], f32, tag="mhat")
                r_col = constp.tile([128, TT], f32, tag="r_col")
                rinv = constp.tile([128, TT], f32, tag="rinv")
                if o["fine_tiles"]:
                    et_tiles = []
                    for _j in range(TT):
                        et_j = bigp.tile([128, KC, 128], bf16,
                                         tag=f"et{_j}")
                        et_tiles.append(et_j)
                    r_tiles = []
                    ri_tiles = []
                    for _j in range(TT):
                        r_j = bigp.tile([128, 1], f32, tag=f"r{_j}")
                        r_tiles.append(r_j)
                        ri_j = bigp.tile([128, 1], f32, tag=f"ri{_j}")
                        ri_tiles.append(ri_j)
                else:
                    et = bigp.tile([128, KC, T], bf16, tag="et")

                # ---------------- phase 1: per row-tile ----------------------
                def do_mm2_epilogue(j, q2c_bc):
                    cj = c_tile(j)
                    pc = ps_mm2.tile([128, D], f32, tag="ps_mm2")
                    for ik in range(KC):
                        lhs_mm2 = (et_tiles[j][:, ik, :] if o["fine_tiles"]
                                   else et[:, ik, j * 128:(j + 1) * 128])
                        nc.tensor.matmul(pc[:], lhs_mm2, q_bf[:, ik],
                                         start=(ik == 0), stop=(ik == KC - 1))
                    if q2c_bc is None:
                        o_t = outp.tile([128, 1024], f32, tag="o23")
                        if o["act_copies"] == "dve":
                            nc.vector.tensor_scalar_mul(o_t[:, 0:512], pc[:],
                                                        (ri_tiles[j][:] if o["fine_tiles"] else rinv[:, j:j + 1]))
                        else:
                            nc.scalar.mul(o_t[:, 0:512], pc[:],
                                          (ri_tiles[j][:] if o["fine_tiles"]
                                           else rinv[:, j:j + 1]))
                        mul_e = (nc.gpsimd if o["mul_eng"] == "gpsimd"
                                 else nc.vector)
                        mul_e.tensor_mul(o_t[:, 512:1024], cj[:],
                                         o_t[:, 0:512])
                        if o["dup_dve"]:
                            nc.vector.tensor_mul(o_t[:, 512:1024], cj[:],
                                                 o_t[:, 0:512])
                        out_dma(j, slice(512, 1536), o_t[:])
                    else:
                        o_t = outp.tile([128, 1536], f32, tag="o234")
                        if o["act_copies"] == "dve":
                            nc.vector.tensor_scalar_mul(o_t[:, 0:512], pc[:],
                                                        (ri_tiles[j][:] if o["fine_tiles"] else rinv[:, j:j + 1]))
                        else:
                            nc.scalar.mul(o_t[:, 0:512], pc[:],
                                          (ri_tiles[j][:] if o["fine_tiles"]
                                           else rinv[:, j:j + 1]))
                        nc.vector.tensor_mul(o_t[:, 512:1024], cj[:],
                                             o_t[:, 0:512])
                        nc.vector.tensor_mul(o_t[:, 1024:1536], cj[:],
                                             q2c_bc[:])
                        out_dma(j, slice(512, 2048), o_t[:])

                for j in range(TT):
                    cj = c_tile(j)  # [128, 512] fp32

                    # C^T for this tile
                    pt = ps_tr.tile([128, 512], f32, tag="ps_tr")
                    for k in range(KC):
                        nc.tensor.transpose(
                            pt[:, k * 128:(k + 1) * 128],
                            cj[:, k * 128:(k + 1) * 128], ident_f[:])
                    ct = workp.tile([128, 512], f32r, tag="ct")
                    if o["ct_eng"] == "act" or (o["alt_copies"] and j % 2 == 0):
                        nc.scalar.copy(ct[:], pt[:])
                    else:
                        nc.vector.tensor_copy(ct[:], pt[:])
                    if o["dup_dve"]:
                        nc.vector.tensor_copy(ct[:], pt[:])

                    # mm1: s' = c @ qa + 1*sq
                    ps = ps_s.tile([128, I], f32, tag="ps_s")
                    if o["dup_pe"]:
                        for k in range(KC):
                            nc.tensor.matmul(
                                ps[:], ct[:, k * 128:(k + 1) * 128],
                                qa[:, k], start=(k == 0), stop=False,
                                skip_group_check=True)
                        for k in range(KC):
                            nc.tensor.matmul(
                                ps[:], ct[:, k * 128:(k + 1) * 128],
                                qa[:, k], start=(k == 0), stop=False,
                                skip_group_check=True)
                    else:
                        for k in range(KC):
                            nc.tensor.matmul(
                                ps[:], ct[:, k * 128:(k + 1) * 128],
                                qa[:, k], start=(k == 0), stop=False)
                    nc.tensor.matmul(ps[:], ones_row[:], sq_row[:],
                                     start=False, stop=True)

                    nc.vector.reduce_max(mhat[:, j:j + 1], ps[:], axis=AX.X)

                    e_tile = workp.tile([128, I], bf16, tag="e")
                    r_dst = (r_tiles[j][:] if o["fine_tiles"]
                             else r_col[:, j:j + 1])
                    if o["exp_accum"]:
                        nc.scalar.activation(e_tile[:], ps[:], AF.Exp,
                                             accum_out=r_dst)
                    else:
                        nc.scalar.activation(e_tile[:], ps[:], AF.Exp)
                        nc.vector.reduce_sum(r_dst, e_tile[:], axis=AX.X)
                    if o["dup_act"]:
                        nc.scalar.activation(e_tile[:], ps[:], AF.Exp,
                                             accum_out=r_col[:, j:j + 1])
                    if o["fine_tiles"]:
                        nc.vector.reciprocal(ri_tiles[j][:], r_tiles[j][:])
                    elif o["batch_recip"]:
                        if j % 4 == 3:
                            nc.vector.reciprocal(rinv[:, j - 3:j + 1],
                                                 r_col[:, j - 3:j + 1])
                    else:
                        nc.vector.reciprocal(rinv[:, j:j + 1],
                                             r_col[:, j:j + 1])

                    # E^T into et[:, ik, j*128:...]
                    pe = ps_tr.tile([128, 512], bf16, tag="ps_tr")
                    for ik in range(KC):
                        nc.tensor.transpose(
                            pe[:, ik * 128:(ik + 1) * 128],
                            e_tile[:, ik * 128:(ik + 1) * 128], ident_b[:])
                    et_dst = (et_tiles[j][:] if o["fine_tiles"]
                              else et[:, :, j * 128:(j + 1) * 128])
                    if o["alt_copies"] and j % 2 == 1:
                        nc.vector.tensor_copy(
                            et_dst, pe[:].rearrange("p (a b) -> p a b", a=KC))
                    else:
                        copy_op(et_dst,
                                pe[:].rearrange("p (a b) -> p a b", a=KC))

                    for _b in range(o["bloat"]):
                        nc.vector.memset(scratch1[0:1, 0:1], 0.0)

                    if o["q2c_inline"]:
                        nc.scalar.activation(g[:, j:j + 1], mhat[:, j:j + 1],
                                             AF.Exp)
                        nc.tensor.matmul(psq2c[:], g[:, j:j + 1], cj[:],
                                         start=(j == 0), stop=(j == TT - 1),
                                         skip_group_check=True)
                        nc.tensor.matmul(psZ[:], g[:, j:j + 1], ones_col[:],
                                         start=(j == 0), stop=(j == TT - 1),
                                         skip_group_check=True)

                    # c block can go out as soon as loaded
                    if not o["early_cout"]:
                        out_dma(j, slice(0, 512), cj[:])

                    if not o["two_pass"]:
                        do_mm2_epilogue(j, None)

                # ---------------- phase 2: q2c -------------------------------
                if not o["q2c_inline"]:
                    gsum = constp.tile([128, 1], f32, tag="gsum")
                    if o["g_accum"]:
                        nc.scalar.activation(g[:], mhat[:], AF.Exp,
                                             accum_out=gsum[:])
                    else:
                        nc.scalar.activation(g[:], mhat[:], AF.Exp)
                        nc.vector.reduce_sum(gsum[:], g[:], axis=AX.X)
                    psZ = ps_s.tile([1, 1], f32, tag="ps_s")
                    nc.tensor.matmul(psZ[:], ones_col[:], gsum[:],
                                     start=True, stop=True)
                    psq2c = ps_s.tile([1, D], f32, tag="ps_s")
                    for j in range(TT):
                        nc.tensor.matmul(psq2c[:], g[:, j:j + 1], c_tile(j),
                                         start=(j == 0), stop=(j == TT - 1))
                Zinv = constp.tile([1, 1], f32, tag="Zinv")
                nc.vector.reciprocal(Zinv[:], psZ[:])
                q2c_row = constp.tile([1, D], f32, tag="q2c_row")
                nc.vector.tensor_scalar_mul(q2c_row[:], psq2c[:], Zinv[:])

                psbc = ps_s.tile([128, D], f32, tag="ps_s")
                nc.tensor.matmul(psbc[:], ones_row_f[:], q2c_row[:],
                                 start=True, stop=True)
                q2c_bc = constp.tile([128, D], f32, tag="q2c_bc")
                copy_op(q2c_bc[:], psbc[:])

                # ---------------- phase 3 ------------------------------------
                if o["two_pass"]:
                    for j in range(TT):
                        do_mm2_epilogue(j, q2c_bc)
                else:
                    for j in range(TT):
                        jj, jr = divmod(j, 4)
                        if o["o4_split"]:
                            mul_e4 = nc.gpsimd if j % 2 else nc.vector
                        else:
                            mul_e4 = (nc.gpsimd if o["mul_eng"] == "gpsimd"
                                      else nc.vector)
                        o4 = outp.tile([128, D], f32, tag="o4")
                        mul_e4.tensor_mul(o4[:], c_tile(j), q2c_bc[:])
                        out_dma(j, slice(1536, 2048), o4[:])

        if timing_mode:
            with tc.tile_pool(name="tickp", bufs=1) as tickp:
                tk = tickp.tile([1, 1], f32, tag="tick")
                nc.vector.memset(tk[:], 1.0)
                nc.sync.dma_start(tick_d[:], tk[:])

    nc.compile()
    return nc


def _get_built():
    global _BUILT
    if _BUILT is None:
        _BUILT = _build()
    return _BUILT


def kernel(c, q, w_c, b_c, w_q, b_q, w_cq, b_cq):
    """Full inputs in, full output out. Data-parallel over batch on 8 cores.

    Biases cancel mathematically (softmax shift invariance), so b_* are
    accepted but unused.
    """
    from concourse import bass_utils

    nc = _get_built()
    c = np.ascontiguousarray(np.asarray(c, dtype=np.float32))
    q = np.ascontiguousarray(np.asarray(q, dtype=np.float32))
    wc = np.ascontiguousarray(np.asarray(w_c, dtype=np.float32))
    wq = np.ascontiguousarray(np.asarray(w_q, dtype=np.float32))
    wcq = np.ascontiguousarray(np.asarray(w_cq, dtype=np.float32))

    in_maps = [
        {"c": c[b], "q": q[b], "wc": wc, "wq": wq, "wcq": wcq}
        for b in range(N_CORES)
    ]
    res = bass_utils.run_bass_kernel_spmd(
        nc, in_maps, core_ids=list(range(N_CORES)))
    return np.stack([res.results[b]["out"] for b in range(N_CORES)])

